# revision 15
# baseline (speedup 1.0000x reference)
"""Trainium2 Bass kernel for nn_DiagonalFunc (64 parallel 2-layer MLPs).

Computation (per batch row b, branch i):
    inp  = concat(x[b, i], z[b, :])                       # 65 features
    h    = inp @ W1[i] + b1[i]                            # [256]
    out  = sum(elu(h) * W2[i]) + b2[i]                    # scalar

Mapping (per core, batch-sharded 8192/8 = 1024 rows):
  - Layer 1 on TensorE as f32r matmuls: stationary = per-(branch, hidden-chunk)
    [128 x 128] weight block (rows 0-63: z-part of W1, row 64+i: x-row of W1,
    rest zero); moving = shared ZX tile [128 rows = z^T | x^T, 512 batch cols].
    PSUM tile [128 hidden, 1024] holds both 512-batch halves of one
    (branch, chunk).
  - ScalarE drains PSUM with Exp(+b1 bias) -> e in SBUF.
  - VectorE runs a custom fused DVE op: u = relu(h+b1) + min(e,1) - 1 = elu(h+b1)
    in ONE 1x pass over PSUM.
  - Layer 2 on TensorE: fp32 matmuls, stationary = W2 column [128,1], moving = u
    [128, 512]; M=1 output lands on psum partition 32*j via col-tiling
    (tile_position), 4 branches per psum bank group; hc chunks accumulate.
    The 8 matmuls of one branch-group are emitted back-to-back so different
    col-groups overlap in the PE array.
  - ScalarE drains the [128, 1024] group-output psum with Identity(+b2 bias);
    DMA gathers rows {0,32,64,96} to the DRAM output.
"""
import numpy as np

import concourse.bacc as bacc
import concourse.tile as tile
from concourse import mybir
from concourse.bass_utils import run_bass_kernel_spmd
import concourse.dve_ops as dve_ops
from concourse.dve_spec import Spec, Src0, Src1, C0, One, relu, minn
from concourse.dve_spec import lower as dve_lower, _has_src1
from concourse.dve_uop import DveOpSpec

# ---------------- problem constants (hardcoded per contract) ----------------
N_CORES = 8
BATCH = 8192
N_BR = 64          # branches
IN_F = 65          # per-branch input features (1 x + 64 z)
HID = 256          # hidden units -> 2 chunks of 128
B_CORE = BATCH // N_CORES   # 1024
F32 = mybir.dt.float32
F32R = mybir.dt.float32r
F16 = mybir.dt.float16

# ---------------- custom DVE op: elu from (h, exp(h+b1)) ----------------
def _elu_ref(in0, in1, s0, s1, imm2):
    h = in0.astype(np.float32) + s0
    return (np.maximum(h, 0) + np.minimum(in1.astype(np.float32), 1.0)
            - 1.0).astype(np.float32)


def _register_elu_op():
    name = "ELU_FE_ANT"
    if name in dve_ops._SUB_OPCODE_FOR_NAME:
        for op in dve_ops.OPS:
            if op.name == name:
                return op
    spec = Spec(body=relu(Src0 + C0) + minn(Src1, One) - One, reference=_elu_ref)
    opcode = max(dve_ops._SUB_OPCODE_FOR_NAME.values()) + 1
    assert opcode < 0x20
    shas = {}
    for ver in ("v3", "v4"):
        try:
            probe = DveOpSpec(name=name, opcode=opcode,
                              uops=dve_lower(spec, ver=ver),
                              rd1_en=_has_src1(spec))
            shas[ver] = probe.sha(ver)
        except Exception:
            pass
    op = dve_ops.DveOp(name, spec, subdim=False, uops_sha=shas)
    dve_ops.OPS.append(op)
    dve_ops.CUSTOM_DVE_SPECS[name] = spec
    dve_ops._SUB_OPCODE_FOR_NAME[name] = opcode
    return op


ELU_OP = _register_elu_op()

# ---------------- program build (cached) ----------------
_NC_CACHE = {}


def _build_nc(loop_n=1):
    if loop_n in _NC_CACHE:
        return _NC_CACHE[loop_n]
    nc = bacc.Bacc("TRN2", target_bir_lowering=False, debug=False,
                   num_devices=N_CORES)
    zx_d = nc.dram_tensor("zx", [128, B_CORE], F32R, kind="ExternalInput").ap()
    wst_d = nc.dram_tensor("wst", [128, N_BR * 2 * 128], F32R,
                           kind="ExternalInput").ap()
    b1_d = nc.dram_tensor("b1t", [128, N_BR * 2], F32, kind="ExternalInput").ap()
    w2_d = nc.dram_tensor("w2t", [128, N_BR * 2], F16, kind="ExternalInput").ap()
    b2_d = nc.dram_tensor("b2t", [128, 16], F32, kind="ExternalInput").ap()
    out_d = nc.dram_tensor("out", [B_CORE, N_BR], F32, kind="ExternalOutput").ap()

    Exp = mybir.ActivationFunctionType.Exp
    Ident = mybir.ActivationFunctionType.Identity

    with tile.TileContext(nc) as tc:
        with tc.tile_pool(name="const", bufs=1) as constp, \
             tc.tile_pool(name="wst", bufs=16) as wstp, \
             tc.tile_pool(name="epool", bufs=4) as epool, \
             tc.tile_pool(name="upool", bufs=20) as upool, \
             tc.tile_pool(name="osb", bufs=4) as osbp, \
             tc.tile_pool(name="psL1", bufs=3, space="PSUM") as psL1, \
             tc.tile_pool(name="psOut", bufs=2, space="PSUM") as psOut:

            zx = constp.tile([128, B_CORE], F32R, tag="zx")
            b1 = constp.tile([128, N_BR * 2], F32, tag="b1")
            w2 = constp.tile([128, N_BR * 2], F16, tag="w2")
            b2 = constp.tile([128, 16], F32, tag="b2")
            # One DMA queue (splitting across engine queues measured 6x
            # slower). Order so the first unit's dependencies land first:
            # zx half 0, group-0 weights, b1 bias, then the rest.
            wst_tiles = [wstp.tile([128, 1024], F32R, tag="wst",
                                   name=f"wst{g}") for g in range(16)]
            nc.sync.dma_start(zx[:, 0:512], zx_d[:, 0:512])
            nc.sync.dma_start(wst_tiles[0][:, 0:256], wst_d[:, 0:256])
            nc.sync.dma_start(b1[:], b1_d[:])
            nc.sync.dma_start(zx[:, 512:1024], zx_d[:, 512:1024])
            nc.sync.dma_start(wst_tiles[0][:, 256:1024], wst_d[:, 256:1024])
            nc.sync.dma_start(w2[:], w2_d[:])
            nc.sync.dma_start(b2[:], b2_d[:])
            for g in range(1, 16):
                nc.sync.dma_start(wst_tiles[g][:],
                                  wst_d[:, 1024 * g:1024 * (g + 1)])

            def emit_l2_and_drain(g, us):
                """Layer-2 cluster + psum-out drain + output DMA for group g.

                Emitted one group late so the PE stream never blocks the
                L1 matmuls that feed ACT/DVE (the critical chain).
                """
                for bc in range(2):
                    pout = psOut.tile([128, 512], F32, tag="pout")
                    for hc in range(2):
                        for j in range(4):
                            jc = 2 * (4 * g + j) + hc
                            u = us[(j, hc)]
                            nc.tensor.matmul(
                                pout[32 * j:32 * j + 1, :],
                                w2[:, jc:jc + 1],
                                u[:, 512 * bc:512 * (bc + 1)],
                                start=(hc == 0), stop=(hc == 1),
                                tile_position=(0, 32 * j))
                    osb = osbp.tile([128, 512], F32, tag="osb")
                    nc.scalar.activation(osb[:], pout[:], Ident,
                                         bias=b2[:, g:g + 1])
                    for j in range(4):
                        br = 4 * g + j
                        nc.sync.dma_start(
                            out_d[512 * bc:512 * (bc + 1), br:br + 1],
                            osb[32 * j:32 * j + 1, :])

            def body(_iv=None):
                pending = None
                for g in range(16):
                    wg = wst_tiles[g]
                    us = {}
                    for j in range(4):
                        br = 4 * g + j
                        for hc in range(2):
                            jc = 2 * br + hc
                            loc = (2 * j + hc) * 128  # col offset inside wg
                            P = psL1.tile([128, 1024], F32, tag="psl1")
                            nc.tensor.matmul(P[:, 0:512], wg[:, loc:loc + 128],
                                             zx[:, 0:512], start=True, stop=True)
                            nc.tensor.matmul(P[:, 512:1024],
                                             wg[:, loc:loc + 128],
                                             zx[:, 512:1024],
                                             start=True, stop=True)
                            e = epool.tile([128, 1024], F32, tag="e")
                            nc.scalar.activation(e[:], P[:], Exp,
                                                 bias=b1[:, jc:jc + 1])
                            u = upool.tile([128, 1024], F16, tag="u")
                            nc.vector._custom_dve(ELU_OP, out=u[:], in0=P[:],
                                                  in1=e[:], s0=b1[:, jc:jc + 1])
                            us[(j, hc)] = u
                    if pending is not None:
                        emit_l2_and_drain(*pending)
                    pending = (g, us)
                emit_l2_and_drain(*pending)

            if isinstance(loop_n, tuple):
                n_iter, n_body = loop_n
            else:
                n_iter, n_body = loop_n, 1
            if n_iter == 1:
                for _ in range(n_body):
                    body()
            else:
                with tc.For_i(0, n_iter, 1):
                    for _ in range(n_body):
                        body()
    nc.compile()
    _NC_CACHE[loop_n] = nc
    return nc


# ---------------- host-side prep + entry point ----------------
def _prep_shared(W1, b1, W2, b2):
    """Host-side rearrangement of the (replicated) weights."""
    W1 = np.asarray(W1, dtype=np.float32)
    b1 = np.asarray(b1, dtype=np.float32)
    W2 = np.asarray(W2, dtype=np.float32)
    b2 = np.asarray(b2, dtype=np.float32)
    # wst [128 rows, 64br * 2hc * 128] ; col-block index = br*2 + hc ordered
    # within groups: block (g, j, hc) lives at 1024*g + (2*j+hc)*128
    wst = np.zeros((128, N_BR * 2 * 128), dtype=np.float32)
    b1t = np.zeros((128, N_BR * 2), dtype=np.float32)
    w2t = np.zeros((128, N_BR * 2), dtype=np.float16)
    for br in range(N_BR):
        g, j = divmod(br, 4)
        for hc in range(2):
            off = 1024 * g + (2 * j + hc) * 128
            wst[0:64, off:off + 128] = W1[br, 1:65, 128 * hc:128 * (hc + 1)]
            wst[64 + br, off:off + 128] = W1[br, 0, 128 * hc:128 * (hc + 1)]
            jc = 2 * br + hc
            b1t[:, jc] = b1[br, 128 * hc:128 * (hc + 1)]
            w2t[:, jc] = W2[br, 128 * hc:128 * (hc + 1)]
    b2t = np.zeros((128, 16), dtype=np.float32)
    for g in range(16):
        for j in range(4):
            b2t[32 * j, g] = b2[4 * g + j]
    return wst, b1t, w2t, b2t


def kernel(x, z, W1, b1, W2, b2):
    x = np.asarray(x, dtype=np.float32)
    z = np.asarray(z, dtype=np.float32)
    wst, b1t, w2t, b2t = _prep_shared(W1, b1, W2, b2)
    nc = _build_nc()
    in_maps = []
    for c in range(N_CORES):
        sl = slice(c * B_CORE, (c + 1) * B_CORE)
        zx = np.concatenate([z[sl].T, x[sl].T], axis=0).astype(np.float32)
        zx = np.ascontiguousarray(zx)
        in_maps.append({"zx": zx, "wst": wst, "b1t": b1t, "w2t": w2t,
                        "b2t": b2t})
    res = run_bass_kernel_spmd(nc, in_maps, list(range(N_CORES)))
    out = np.concatenate([res.results[c]["out"] for c in range(N_CORES)],
                         axis=0)
    return out.astype(np.float32)
